# revision 1
# baseline (speedup 1.0000x reference)
# GCN encoder (DGI) forward on 8 Trainium2 NeuronCores.
#
# Node-partitioned (graph-parallel) sharding:
#   - nodes are split contiguously across the 8 cores (N/8 per core)
#   - each core owns the edges whose *target* lands in its node range
#   - phase 1: every core computes xw' = dinv[s] * (x_s @ W_sn) for its own
#     nodes, then an AllGather replicates the full xw' table to every core
#   - phase 2: each core gathers source rows for its edges with bulk indirect
#     DMA, scatter-adds them into per-window PSUM accumulators with one-hot
#     selector matmuls on the PE, and applies dinv[t]/bias/PReLU in the
#     epilogue.
#
# Host-side work is limited to index preprocessing (edge routing/sorting,
# degree counting) and the tiny spectral-norm power iteration on W.

import numpy as np

import concourse.bacc as bacc
import concourse.bass as bass
import concourse.mybir as mybir
import concourse.tile as tile
from concourse.bass_utils import run_bass_kernel_spmd
from concourse.masks import make_identity

P = 128
F32 = mybir.dt.float32
I32 = mybir.dt.int32

# test-harness hooks (ignored in grading): set TRACE=True before calling
# kernel() to capture an NTFF profile; the BassKernelResults lands in
# LAST_RESULT.
TRACE = False
LAST_RESULT = None


def _l2n(v, eps=1e-12):
    return v / (np.linalg.norm(v) + eps)


def _spectral_norm_host(W, u):
    W = W.astype(np.float32)
    u = u.astype(np.float32)
    v = _l2n(W.T @ u)
    u2 = _l2n(W @ v)
    sigma = np.float32(u2 @ (W @ v))
    return W / sigma


def _prep_host(x, edge_index, n_cores, win_group, bucket_rows, max_call_chunks=0):
    """Route edges to cores by target and build the SPMD chunk schedule.

    Chunks are 128 edges, each mapping into one 128-target window and one
    source bucket (dma_gather has int16 indices, so the gathered table is
    addressed in buckets of `bucket_rows` rows).  Chunk order: for each
    super-group of `win_group` windows, for each bucket, the chunks of the
    group's windows.  One dma_gather call covers one (group, bucket) run.
    Self-loops are NOT in the edge stream (added in the epilogue).
    """
    n, nfeat = x.shape
    assert n % n_cores == 0
    npc = n // n_cores
    nwin = -(-npc // P)
    nbuck = -(-n // bucket_rows)
    assert bucket_rows < 32768

    row = np.ascontiguousarray(edge_index[0]).astype(np.int64)
    col = np.ascontiguousarray(edge_index[1]).astype(np.int64)

    # sort all edges by (target window, source bucket) so each (core, window,
    # bucket) run is contiguous; target order within a chunk is free (tloc).
    wkey = (col // npc) * nwin + (col % npc) // P  # global window id
    key = wkey * nbuck + row // bucket_rows
    order = np.argsort(key, kind="stable")
    rs = row[order]
    cs = col[order]
    bs = rs // bucket_rows
    cwb_sorted = key[order]

    deg = 1.0 + np.bincount(col, minlength=n).astype(np.float64)  # + self loop
    dinv_all = (deg ** -0.5).astype(np.float32)

    # counts per (core, window, bucket)
    cnt = np.bincount(key, minlength=n_cores * nwin * nbuck).reshape(
        n_cores, nwin, nbuck
    )
    kwb = -(-cnt // P)  # chunks per (c, w, b)
    kwb = kwb.max(axis=0)  # [nwin, nbuck] shared schedule

    # chunk order + gather-call runs
    chunk_win = []
    chunk_bucket = []
    call_sizes = []  # chunks per dma_gather call
    for wg in range(0, nwin, win_group):
        ws = range(wg, min(wg + win_group, nwin))
        for b in range(nbuck):
            r = int(sum(kwb[w, b] for w in ws))
            if r == 0:
                continue
            if max_call_chunks > 0:
                q = r
                while q > 0:
                    call_sizes.append(min(q, max_call_chunks))
                    q -= max_call_chunks
            else:
                call_sizes.append(r)
            for w in ws:
                chunk_win.extend([w] * kwb[w, b])
                chunk_bucket.extend([b] * kwb[w, b])
    chunk_win = np.asarray(chunk_win)
    chunk_bucket = np.asarray(chunk_bucket)
    nchunks = len(chunk_win)

    # first/last chunk per window in this order
    first_of_win = np.zeros(nchunks, bool)
    last_of_win = np.zeros(nchunks, bool)
    seen = set()
    for j in range(nchunks):
        w = int(chunk_win[j])
        if w not in seen:
            first_of_win[j] = True
            seen.add(w)
    seen = set()
    for j in range(nchunks - 1, -1, -1):
        w = int(chunk_win[j])
        if w not in seen:
            last_of_win[j] = True
            seen.add(w)

    # first destination chunk per (w, b)
    base_by_wb = {}
    for j in range(nchunks):
        key2 = (int(chunk_win[j]), int(chunk_bucket[j]))
        if key2 not in base_by_wb:
            base_by_wb[key2] = j

    # segment boundaries of (core, window, bucket) runs in the sorted list
    seg_lo_idx = np.searchsorted(
        cwb_sorted, np.arange(n_cores * nwin * nbuck), side="left"
    )
    seg_hi_idx = np.searchsorted(
        cwb_sorted, np.arange(n_cores * nwin * nbuck), side="right"
    )

    src_cores = []
    tloc_cores = []
    dinv_cores = []
    for c in range(n_cores):
        src_flat = np.zeros(nchunks * P, np.int16)
        tloc_flat = np.full(nchunks * P, -1.0, np.float32)
        for w in range(nwin):
            for b in range(nbuck):
                if (w, b) not in base_by_wb:
                    continue
                s = c * nwin * nbuck + w * nbuck + b
                i0, i1 = seg_lo_idx[s], seg_hi_idx[s]
                m = i1 - i0
                if m == 0:
                    continue
                d0 = base_by_wb[(w, b)] * P
                src_flat[d0 : d0 + m] = (rs[i0:i1] - b * bucket_rows).astype(
                    np.int16
                )
                tloc_flat[d0 : d0 + m] = (cs[i0:i1] - c * npc - w * P).astype(
                    np.float32
                )
        # dma_gather idx layout: idx i -> partition i%16, col i//16,
        # replicated over the 8 groups of 16 partitions.
        a = src_flat.reshape(nchunks, 8, 16)  # [j, p//16, p%16]
        a = np.transpose(a, (2, 0, 1)).reshape(16, nchunks * 8)
        src_cores.append(np.ascontiguousarray(np.tile(a, (8, 1))))
        tloc_cores.append(np.ascontiguousarray(tloc_flat.reshape(nchunks, P).T))

        dv = np.zeros(nwin * P, np.float32)
        dv[:npc] = dinv_all[c * npc : (c + 1) * npc]
        dinv_cores.append(np.ascontiguousarray(dv.reshape(nwin, P).T))

    return dict(
        npc=npc,
        nwin=nwin,
        nbuck=nbuck,
        nchunks=nchunks,
        chunk_win=chunk_win,
        chunk_bucket=chunk_bucket,
        call_sizes=call_sizes,
        first_of_win=first_of_win,
        last_of_win=last_of_win,
        src_cores=src_cores,
        tloc_cores=tloc_cores,
        dinv_cores=dinv_cores,
    )


def _build_nc(
    n,
    nfeat,
    nhid,
    n_cores,
    nwin,
    nbuck,
    bucket_rows,
    nchunks,
    chunk_win,
    chunk_bucket,
    call_sizes,
    first_of_win,
    last_of_win,
    alpha,
    gather_bufs=6,
):
    npc_pad = nwin * P
    npc = n // n_cores
    assert nfeat % P == 0
    nk = nfeat // P  # contraction tiles for x @ W

    nc = bacc.Bacc(
        "TRN2",
        target_bir_lowering=False,
        debug=False,
        enable_asserts=False,
        num_devices=n_cores,
        num_swdge_queues=4,
    )

    x_in = nc.dram_tensor("x_sh", [npc_pad, nfeat], F32, kind="ExternalInput")
    w_in = nc.dram_tensor("w_sn", [nfeat, nhid], F32, kind="ExternalInput")
    dinv_in = nc.dram_tensor("dinv", [P, nwin], F32, kind="ExternalInput")
    bias_in = nc.dram_tensor("bias_t", [P, nhid], F32, kind="ExternalInput")
    max_call = max(call_sizes)
    iota_in = nc.dram_tensor("iota_t", [P, max_call * P], F32, kind="ExternalInput")
    src_in = nc.dram_tensor(
        "src_idx", [P, nchunks * 8], mybir.dt.int16, kind="ExternalInput"
    )
    tloc_in = nc.dram_tensor("tloc", [P, nchunks], F32, kind="ExternalInput")
    out_d = nc.dram_tensor("out_sh", [npc_pad, nhid], F32, kind="ExternalOutput")

    assert sum(call_sizes) == nchunks

    with tile.TileContext(nc) as tc:
        with (
            tc.tile_pool(name="consts", bufs=1) as cpool,
            tc.tile_pool(name="dram", bufs=1, space="DRAM") as dpool,
        ):
            # constants
            w_sb = cpool.tile([P, nk, nhid], F32)
            nc.sync.dma_start(
                w_sb[:], w_in[:].rearrange("(k p) h -> p k h", p=P)
            )
            bias_sb = cpool.tile([P, nhid], F32)
            nc.sync.dma_start(bias_sb[:], bias_in[:])
            iota_sb = cpool.tile([P, max_call * P], F32)
            nc.sync.dma_start(iota_sb[:], iota_in[:])
            dinv_sb = cpool.tile([P, nwin], F32)
            nc.sync.dma_start(dinv_sb[:], dinv_in[:])
            ident = cpool.tile([P, P], F32)
            make_identity(nc, ident[:])
            src_sb = cpool.tile([P, nchunks * 8], mybir.dt.int16)
            nc.sync.dma_start(src_sb[:], src_in[:])
            tloc_sb = cpool.tile([P, nchunks], F32)
            nc.sync.dma_start(tloc_sb[:], tloc_in[:])

            ag_in = dpool.tile([npc, nhid], F32)
            ag_out = dpool.tile([n, nhid], F32, addr_space="Shared")

            # ---- phase 1: xw' = dinv[s] * (x_s @ W_sn) for owned nodes ----
            with (
                tc.tile_pool(name="p1x", bufs=3) as xpool,
                tc.tile_pool(name="p1xt", bufs=3) as xtpool,
                tc.tile_pool(name="p1o", bufs=3) as xwpool,
                tc.tile_pool(name="p1pt", bufs=3, space="PSUM") as psumT,
                tc.tile_pool(name="p1pm", bufs=2, space="PSUM") as psumXW,
            ):
                for w in range(nwin):
                    nrow = min(P, npc - w * P)
                    xt = xpool.tile([P, nfeat], F32)
                    nc.sync.dma_start(xt[:], x_in[w * P : (w + 1) * P, :])
                    xT = xtpool.tile([P, nk, P], F32)
                    for k in range(nk):
                        pT = psumT.tile([P, P], F32)
                        nc.tensor.transpose(
                            pT[:], xt[:, k * P : (k + 1) * P], ident[:]
                        )
                        nc.vector.tensor_copy(xT[:, k, :], pT[:])
                    pxw = psumXW.tile([P, nhid], F32)
                    for k in range(nk):
                        nc.tensor.matmul(
                            pxw[:],
                            lhsT=xT[:, k, :],
                            rhs=w_sb[:, k, :],
                            start=(k == 0),
                            stop=(k == nk - 1),
                        )
                    xwp = xwpool.tile([P, nhid], F32)
                    nc.vector.tensor_scalar(
                        out=xwp[:],
                        in0=pxw[:],
                        scalar1=dinv_sb[:, w : w + 1],
                        scalar2=None,
                        op0=mybir.AluOpType.mult,
                    )
                    nc.sync.dma_start(
                        ag_in[w * P : w * P + nrow, :], xwp[:nrow, :]
                    )

            nc.gpsimd.collective_compute(
                "AllGather",
                mybir.AluOpType.bypass,
                replica_groups=[list(range(n_cores))],
                ins=[ag_in[:]],
                outs=[ag_out[:]],
            )

            # ---- phase 2: gather + one-hot matmul scatter-add + epilogue ----
            out_sb = cpool.tile([P, nwin * nhid], F32)
            psum_by_win = {}
            with (
                tc.tile_pool(name="gat", bufs=gather_bufs) as gpool,
                tc.tile_pool(name="sel", bufs=6) as spool,
                tc.tile_pool(name="slf", bufs=4) as lpool,
                tc.tile_pool(name="tmp", bufs=4) as tpool,
                tc.tile_pool(name="acc", bufs=8, space="PSUM") as ppool,
            ):
                j = 0
                for ci, r in enumerate(call_sizes):
                    gbuf = gpool.tile(
                        [P, max_call * nhid], F32, tag="gbuf", name="gbuf"
                    )
                    b = int(chunk_bucket[j])
                    rows = min(bucket_rows, n - b * bucket_rows)
                    nc.gpsimd.dma_gather(
                        gbuf[:, : r * nhid].rearrange("p (k e) -> p k e", e=nhid),
                        ag_out[b * bucket_rows : b * bucket_rows + rows, :],
                        src_sb[:, j * 8 : (j + r) * 8],
                        r * P,
                        r * P,
                        nhid,
                        queue_num=ci % 4,
                    )
                    # one-hot selectors for the whole call in one DVE op
                    sel_big = spool.tile(
                        [P, max_call * P], F32, tag="sel", name="sel_big"
                    )
                    nc.vector.tensor_tensor(
                        out=sel_big[:, : r * P].rearrange(
                            "p (k e) -> p k e", e=P
                        ),
                        in0=tloc_sb[:, j : j + r].to_broadcast([P, r, P]),
                        in1=iota_sb[:, : r * P].rearrange(
                            "p (k e) -> p k e", e=P
                        ),
                        op=mybir.AluOpType.is_equal,
                    )
                    for kk in range(r):
                        w = int(chunk_win[j])
                        if first_of_win[j]:
                            psum_by_win[w] = ppool.tile(
                                [P, nhid], F32, tag="pw", name="pw"
                            )
                        pw = psum_by_win[w]
                        nc.tensor.matmul(
                            pw[:],
                            lhsT=sel_big[:, kk * P : (kk + 1) * P],
                            rhs=gbuf[:, kk * nhid : (kk + 1) * nhid],
                            start=bool(first_of_win[j]),
                            stop=bool(last_of_win[j]),
                        )
                        if last_of_win[j]:
                            # self-loop term: + xw'[own window nodes]
                            sc = lpool.tile([P, nhid], F32, tag="sc", name="sc")
                            nrow = min(P, npc - w * P)
                            if nrow < P:
                                nc.vector.memset(sc[:], 0.0)
                            nc.sync.dma_start(
                                sc[:nrow, :], ag_in[w * P : w * P + nrow, :]
                            )
                            agg = tpool.tile([P, nhid], F32, tag="agg", name="agg")
                            nc.vector.tensor_tensor(
                                out=agg[:],
                                in0=pw[:],
                                in1=sc[:],
                                op=mybir.AluOpType.add,
                            )
                            seg = out_sb[:, w * nhid : (w + 1) * nhid]
                            # dinv[t] * agg on ACT
                            nc.scalar.activation(
                                out=seg,
                                in_=agg[:],
                                func=mybir.ActivationFunctionType.Copy,
                                scale=dinv_sb[:, w : w + 1],
                            )
                            nc.vector.tensor_tensor(
                                out=seg,
                                in0=seg,
                                in1=bias_sb[:],
                                op=mybir.AluOpType.add,
                            )
                            # PReLU(y) = max(y, alpha*y) for 0 <= alpha <= 1
                            t2 = tpool.tile([P, nhid], F32, tag="t2", name="t2")
                            if 0.0 <= alpha <= 1.0:
                                nc.vector.tensor_scalar_mul(t2, seg, float(alpha))
                                nc.vector.tensor_tensor(
                                    out=seg,
                                    in0=seg,
                                    in1=t2,
                                    op=mybir.AluOpType.max,
                                )
                            else:
                                # general: max(y,0) + alpha*min(y,0)
                                nc.vector.tensor_scalar(
                                    out=t2,
                                    in0=seg,
                                    scalar1=0.0,
                                    scalar2=float(alpha),
                                    op0=mybir.AluOpType.min,
                                    op1=mybir.AluOpType.mult,
                                )
                                nc.vector.tensor_scalar_max(seg, seg, 0.0)
                                nc.vector.tensor_tensor(
                                    out=seg,
                                    in0=seg,
                                    in1=t2,
                                    op=mybir.AluOpType.add,
                                )
                        j += 1

            nc.sync.dma_start(
                out_d[:].rearrange("(w p) h -> p w h", p=P),
                out_sb[:].rearrange("p (w h) -> p w h", h=nhid),
            )

    nc.compile()
    return nc


def kernel(**inputs):
    x = np.asarray(inputs["x"], dtype=np.float32)
    edge_index = np.asarray(inputs["edge_index"])
    W = np.asarray(inputs["W"], dtype=np.float32)
    bias = np.asarray(inputs["bias"], dtype=np.float32)
    prelu_a = np.asarray(inputs["prelu_a"], dtype=np.float32)
    u = np.asarray(inputs["u"], dtype=np.float32)

    n, nfeat = x.shape
    nhid = W.shape[1]
    n_cores = 8
    win_group = 4
    nbuck = -(-n // 32767)  # int16 index reach per dma_gather bucket
    bucket_rows = -(-n // nbuck)
    alpha = float(prelu_a.reshape(-1)[0])

    # one dma_gather call must stay under the 1024-descriptor SWDGE ring
    # carveout (dynamic_dma_scratch_size//16); 7 chunks = 896 descriptors
    max_call_chunks = 7

    w_sn = _spectral_norm_host(W, u)
    prep = _prep_host(
        x, edge_index, n_cores, win_group, bucket_rows, max_call_chunks
    )
    npc, nwin, nchunks = prep["npc"], prep["nwin"], prep["nchunks"]

    nc = _build_nc(
        n,
        nfeat,
        nhid,
        n_cores,
        nwin,
        prep["nbuck"],
        bucket_rows,
        nchunks,
        prep["chunk_win"],
        prep["chunk_bucket"],
        prep["call_sizes"],
        prep["first_of_win"],
        prep["last_of_win"],
        alpha,
    )

    bias_t = np.ascontiguousarray(np.tile(bias[None, :], (P, 1)))
    max_call = max(prep["call_sizes"])
    iota_t = np.ascontiguousarray(
        np.tile(
            np.tile(np.arange(P, dtype=np.float32), max_call)[None, :], (P, 1)
        )
    )
    npc_pad = nwin * P

    in_maps = []
    for c in range(n_cores):
        x_sh = np.zeros((npc_pad, nfeat), np.float32)
        x_sh[:npc] = x[c * npc : (c + 1) * npc]
        in_maps.append(
            {
                "x_sh": x_sh,
                "w_sn": w_sn,
                "dinv": prep["dinv_cores"][c],
                "bias_t": bias_t,
                "iota_t": iota_t,
                "src_idx": prep["src_cores"][c],
                "tloc": prep["tloc_cores"][c],
            }
        )

    res = run_bass_kernel_spmd(
        nc, in_maps, core_ids=list(range(n_cores)), trace=TRACE
    )
    global LAST_RESULT
    LAST_RESULT = res
    out = np.concatenate(
        [res.results[c]["out_sh"][:npc] for c in range(n_cores)], axis=0
    )
    return out

